# revision 12
# baseline (speedup 1.0000x reference)
"""Sharded causal attention kernel for trn2, v3.

Sharding: 8 cores = 2 batches x 4 head-groups (4 heads each).
v3 vs v2:
  - all SBUF tensors bf16 (DVE 2x, half DMA); PSUM stays f32
  - software-pipelined: attention for q-block qb runs one chunk behind the
    projections, emission interleaved at key-tile granularity so PE always
    has ready work while ACT churns exp
  - compact rotary weights [D,128] (no zero columns)
  - rs_col via PE transposes of ssq (no diag-extract trick)
  - pv with v-as-moving (65-row matmuls), per-partition softmax normalize,
    DMA-xbar transpose of attn, then out projection
"""

from contextlib import ExitStack

import numpy as np

import concourse.bass as bass
import concourse.mybir as mybir
import concourse.tile as tile
from concourse import bacc
from concourse.bass import _add_dep_helper as add_dep

f32 = mybir.dt.float32
f32r = mybir.dt.float32r
bf16 = mybir.dt.bfloat16
AF = mybir.ActivationFunctionType
OP = mybir.AluOpType

D = 1024
HPC = 4          # heads per core
DH = 64
ROT = 32
P = 128
NEG = -1e30


def build_program(n=2048, use_kmask=False):
    KT = D // P          # 8 contraction tiles
    NCH = n // 512       # 4 token chunks
    NTOK = n // P        # 16 token tiles
    nc = bacc.Bacc("TRN2", target_bir_lowering=False, debug=False)

    def din(name, shape, dt_):
        return nc.dram_tensor(name, shape, dt_, kind="ExternalInput")

    # host packs x/weights t-major so each is one DMA into a [128, ...] tile
    xT_d = din("xT", [P, KT * n], bf16)
    wq_d = din("wq", [P, KT * HPC * DH], bf16)
    wk_d = din("wk", [P, KT * HPC * DH], bf16)
    wv_d = din("wv", [P, KT * HPC * DH], bf16)
    wqr_d = din("wqr", [P, KT * P], bf16)          # compact rot cols [4 heads x 32]
    wkr_d = din("wkr", [P, KT * P], bf16)
    wo_d = din("wo", [HPC * DH, D], bf16)
    cos_d = din("cos128", [P, n], bf16)            # qT-aligned: rows 0:32,64:96 cos
    sin_d = din("sinc", [P, n], bf16)              # compact: row 32h+j = sin_j
    tri_d = din("tri", [P, P], f32)
    id_d = din("ident", [P, P], bf16)
    km_d = din("kmask", [P, NTOK], f32) if use_kmask else None
    out_d = nc.dram_tensor("out", [n, D], bf16, kind="ExternalOutput")

    with tile.TileContext(nc) as tc, ExitStack() as top:
        persist = top.enter_context(tc.tile_pool(name="persist", bufs=1))
        ones_bf = persist.tile([P, 1], bf16, name="ones_bf")
        nc.vector.memset(ones_bf, 1.0)
        ones_row_f = persist.tile([1, P], f32, name="ones_row_f")
        nc.vector.memset(ones_row_f, 1.0)
        ones_row = persist.tile([1, P], f32r, name="ones_row")
        nc.vector.tensor_copy(ones_row, ones_row_f)
        tri_sb = persist.tile([P, P], f32, name="tri_sb")
        ident_sb = persist.tile([P, P], bf16, name="ident_sb")
        km_sb = persist.tile([P, NTOK], f32, name="km_sb") if use_kmask else None

        big = top.enter_context(tc.tile_pool(name="big", bufs=1))
        # x loaded chunk-major: one DMA brings all KT contraction tiles for a
        # 512-token chunk, so chunk-0 compute starts after ~1/4 of the x bytes
        x_all = big.tile([P, KT * n], bf16, name="x_all")
        x_allv = x_all.rearrange("p (t n) -> p t n", t=KT)
        xT_dv = xT_d.rearrange("p (t n) -> p t n", t=KT)
        x_sb = [x_all[:, t * n:(t + 1) * n] for t in range(KT)]
        wq_sb = big.tile([P, KT * HPC * DH], bf16, name="wq")
        wk_sb = big.tile([P, KT * HPC * DH], bf16, name="wk")
        wv_sb = big.tile([P, KT * HPC * DH], bf16, name="wv")
        wqr_sb = big.tile([P, KT * P], bf16, name="wqr")
        wkr_sb = big.tile([P, KT * P], bf16, name="wkr")
        cos_sb = big.tile([P, n], bf16, name="cos_sb")
        sin_sb = big.tile([P, n], bf16, name="sin_sb")
        wo_sb = [big.tile([P, D], bf16, name=f"wo{m}") for m in range(2)]
        # DMA issue order = single-queue service order: schedule each input
        # just before its first consumer needs it
        nc.sync.dma_start(out=x_allv[:, 0:2, 0:512], in_=xT_dv[:, 0:2, 0:512])
        nc.sync.dma_start(out=x_allv[:, 2:4, 0:512], in_=xT_dv[:, 2:4, 0:512])
        nc.sync.dma_start(out=x_allv[:, 4:KT, 0:512], in_=xT_dv[:, 4:KT, 0:512])
        nc.sync.dma_start(out=ident_sb, in_=id_d[:])
        nc.sync.dma_start(out=wq_sb, in_=wq_d[:])
        nc.sync.dma_start(out=wk_sb, in_=wk_d[:])
        nc.sync.dma_start(out=x_allv[:, :, 512:1024], in_=xT_dv[:, :, 512:1024])
        nc.sync.dma_start(out=wqr_sb, in_=wqr_d[:])
        nc.sync.dma_start(out=wkr_sb, in_=wkr_d[:])
        nc.sync.dma_start(out=cos_sb, in_=cos_d[:])
        nc.sync.dma_start(out=sin_sb, in_=sin_d[:])
        nc.sync.dma_start(out=x_allv[:, :, 1024:1536], in_=xT_dv[:, :, 1024:1536])
        nc.sync.dma_start(out=wv_sb, in_=wv_d[:])
        nc.sync.dma_start(out=x_allv[:, :, 1536:2048], in_=xT_dv[:, :, 1536:2048])
        nc.sync.dma_start(out=tri_sb, in_=tri_d[:])
        for m in range(2):
            nc.sync.dma_start(out=wo_sb[m], in_=wo_d[m * P:(m + 1) * P, :])
        if use_kmask:
            nc.sync.dma_start(out=km_sb, in_=km_d[:])
        wq = [wq_sb[:, t * HPC * DH:(t + 1) * HPC * DH] for t in range(KT)]
        wk = [wk_sb[:, t * HPC * DH:(t + 1) * HPC * DH] for t in range(KT)]
        wv = [wv_sb[:, t * HPC * DH:(t + 1) * HPC * DH] for t in range(KT)]
        wqr = [wqr_sb[:, t * P:(t + 1) * P] for t in range(KT)]
        wkr = [wkr_sb[:, t * P:(t + 1) * P] for t in range(KT)]

        qkv = top.enter_context(tc.tile_pool(name="qkv", bufs=1))
        qT = [qkv.tile([P, n], bf16, name=f"qT{m}", tag=f"qT{m}") for m in range(2)]
        kT = [qkv.tile([P, n], bf16, name=f"kT{m}", tag=f"kT{m}") for m in range(2)]
        v_sb = [qkv.tile([P, HPC * (DH + 1)], bf16, name=f"v{tk}", tag=f"v{tk}")
                for tk in range(NTOK)]
        rs_col = qkv.tile([P, NTOK], f32, name="rs_col")
        s_row = qkv.tile([1, n], f32r, name="s_row")
        ssq_sb = qkv.tile([1, n], bf16, name="ssq_sb")

        # PSUM budget (8 banks): pa (prelude + projections) 2,
        # pb (attention ppv accumulators + out-proj po) 2, psim 4
        pa = top.enter_context(tc.tile_pool(name="pa", bufs=2, space="PSUM"))
        pb = top.enter_context(tc.tile_pool(name="pb", bufs=1, space="PSUM"))
        psim = top.enter_context(tc.tile_pool(name="psim", bufs=1, space="PSUM"))
        sqp = top.enter_context(tc.tile_pool(name="sqp", bufs=2))
        rotu = top.enter_context(tc.tile_pool(name="rotu", bufs=2))
        esp = top.enter_context(tc.tile_pool(name="esp", bufs=1))
        atp = top.enter_context(tc.tile_pool(name="atp", bufs=3))
        obp = top.enter_context(tc.tile_pool(name="obp", bufs=3))
        rcp = top.enter_context(tc.tile_pool(name="rcp", bufs=3))

        # ---------------- phase A emitters (chunk c), as a piece list ----
        prelude_acts = []  # Sqrt instructions that must precede the first Exp

        def prelude(c):
            """rmsnorm stats for chunk c; all ACT Sqrt/Copy happen before the
            first Exp so the activation table loads exactly twice."""
            csl = slice(c * 512, (c + 1) * 512)
            ssq = pb.tile([1, 512], f32, name=f"ssq{c}", tag="ppv")
            for t in range(KT):
                sq = sqp.tile([P, 512], bf16, name=f"sq{t}_{c}", tag="sq")
                # spread squares across DVE/ACT/Pool (Square is in every act
                # table set; Pool is SBUF-only) to keep startup queues short
                if c == 0:
                    nc.vector.tensor_mul(sq, x_sb[t][:, csl], x_sb[t][:, csl])
                elif c == 2:
                    nc.scalar.activation(sq, x_sb[t][:, csl], AF.Square)
                else:
                    nc.gpsimd.tensor_mul(sq, x_sb[t][:, csl], x_sb[t][:, csl])
                nc.tensor.matmul(ssq, ones_bf, sq,
                                 start=(t == 0), stop=(t == KT - 1))
            # sqrt(ssq/D) then reciprocal (row form for bc broadcast)
            my_acts = []
            my_acts.append(
                nc.scalar.activation(s_row[:, csl], ssq, AF.Sqrt, scale=1.0 / D))
            with nc.allow_low_precision(reason="f32r is f32-width"):
                nc.vector.reciprocal(s_row[:, csl], s_row[:, csl])
            my_acts.append(
                nc.scalar.activation(ssq_sb[:, csl], ssq, AF.Copy))
            # rs_col tiles via PE transpose of ssq (bf16: f32r transpose is
            # broken in walrus codegen)
            rst = pb.tile([P, 8], bf16, name=f"rst{c}", tag="ppv")
            rstv = rst.rearrange("p (a b) -> p a b", b=2)
            for tb in range(4):
                tk = c * 4 + tb
                # even columns only: PSUM accesses must be 4-byte aligned
                nc.tensor.transpose(rstv[:, tb, 0:1],
                                    ssq_sb[:, tk * P:(tk + 1) * P],
                                    ones_bf[0:1, 0:1])
            my_acts.append(
                nc.scalar.activation(rs_col[:, c * 4:c * 4 + 4], rstv[:, :, 0],
                                     AF.Sqrt, scale=1.0 / D))
            if c < NCH - 1:
                # c3's stats come late (Pool squares); gating every exp on it
                # would stall attention — let it float and eat one table reload
                prelude_acts.extend(my_acts)
            nc.vector.reciprocal(rs_col[:, c * 4:c * 4 + 4],
                                 rs_col[:, c * 4:c * 4 + 4])
            # broadcast rs over partitions, fold into cos/sin
            bc = pb.tile([P, 512], f32, name=f"bc{c}", tag="ppv")
            nc.tensor.matmul(bc, ones_row, s_row[:, csl],
                             start=True, stop=True)
            nc.vector.tensor_mul(cos_sb[:, csl], cos_sb[:, csl], bc)
            nc.vector.tensor_mul(sin_sb[:, csl], sin_sb[:, csl], bc)

        def qk_pieces(c, which):
            csl = slice(c * 512, (c + 1) * 512)
            base, wmain, wrot, nm_ = ((qT, wq, wqr, "q") if which == "q"
                                      else (kT, wk, wkr, "k"))

            def p_m(m):
                ps = pa.tile([P, 512], f32, name=f"p{nm_}{m}_{c}", tag="pa")
                for t in range(KT):
                    nc.tensor.matmul(ps, wmain[t][:, m * P:(m + 1) * P],
                                     x_sb[t][:, csl],
                                     start=(t == 0), stop=(t == KT - 1))
                nc.vector.tensor_mul(base[m][:, csl], ps, cos_sb[:, csl])

            def p_rot():
                # psr partition layout (wqr col order [h0|h2|h1|h3]):
                # m=0 reads rows 0:96 (h0,-,h1), m=1 rows 32:128 (h2,-,h3);
                # u tiles land base-aligned with qT rot rows {0:32, 64:96} so
                # the SBUF-SBUF adds have equal base partitions (hw rule)
                psr = pa.tile([P, 512], f32, name=f"p{nm_}r_{c}", tag="pa")
                for t in range(KT):
                    nc.tensor.matmul(psr, wrot[t], x_sb[t][:, csl],
                                     start=(t == 0), stop=(t == KT - 1))
                for m in range(2):
                    u = rotu.tile([P, 512], bf16, name=f"u_{nm_}{m}_{c}", tag="u")
                    if m == 0:
                        nc.vector.tensor_mul(u[0:96, :], psr[0:96, :],
                                             sin_sb[0:96, csl])
                    else:
                        # aligned partition windows: <=32 parts from base 32/96
                        nc.vector.tensor_mul(u[0:32, :], psr[32:64, :],
                                             sin_sb[0:32, csl])
                        nc.vector.tensor_mul(u[64:96, :], psr[96:128, :],
                                             sin_sb[64:96, csl])
                    for h2 in range(2):
                        bsl = base[m][64 * h2:64 * h2 + 32, csl]
                        usl = u[64 * h2:64 * h2 + 32, :]
                        if c == 0:
                            nc.vector.tensor_tensor(bsl, bsl, usl, OP.add)
                        else:
                            nc.gpsimd.tensor_tensor(bsl, bsl, usl, OP.add)

            return [lambda: p_m(0), lambda: p_m(1), p_rot]

        def v_pieces(c):
            out = []
            for tb in range(4):
                tk = c * 4 + tb

                def p_v(tk=tk):
                    pv = pa.tile([P, HPC * DH], f32, name=f"pv_{tk}", tag="pa")
                    for t in range(KT):
                        nc.tensor.matmul(pv, x_sb[t][:, tk * P:(tk + 1) * P],
                                         wv[t], start=(t == 0), stop=(t == KT - 1))
                    vv = v_sb[tk].rearrange("p (h c2) -> p h c2", h=HPC)
                    nc.vector.tensor_scalar_mul(
                        vv[:, :, 0:DH], pv.rearrange("p (h c2) -> p h c2", h=HPC),
                        rs_col[:, tk:tk + 1])
                    for hh in range(HPC):
                        nc.gpsimd.tensor_copy(vv[:, hh, DH:DH + 1], ones_bf)

                out.append(p_v)
            return out

        # ---------------- attention emitter for q-block qb -----------------
        def emit_sims(qb, kt, Es, tagp=""):
            """sims + mask + exp for one key tile (all 4 heads in one 4-bank
            psum tile -> single exp op per off-diag tile)."""
            d = kt - 4 * qb
            lo = max(0, d) * P  # q cols < lo are strictly above the diag
            E = esp.tile([P, 2048], bf16, name=f"E{tagp}_{kt}",
                         tag=f"E{tagp}_{kt}")
            for pr in range(2):
                sim = psim.tile([P, 1024], f32, name=f"s{tagp}{pr}_{qb}_{kt}",
                                tag=f"sim{pr}")
                for h2 in range(2):
                    nc.tensor.matmul(
                        sim[:, 512 * h2 + lo:512 * h2 + 512],
                        kT[pr][64 * h2:64 * h2 + 64, kt * P:(kt + 1) * P],
                        qT[pr][64 * h2:64 * h2 + 64,
                               qb * 512 + lo:(qb + 1) * 512],
                        start=True, stop=True, tile_position=(64 * h2, 0))
                if d >= 0:
                    for h2 in range(2):
                        sl = sim[:, 512 * h2 + d * P:512 * h2 + (d + 1) * P]
                        nc.vector.tensor_tensor(sl, sl, tri_sb, OP.add)
                if use_kmask:
                    for h2 in range(2):
                        sl = sim[:, 512 * h2:512 * h2 + 512]
                        nc.vector.tensor_scalar_add(sl, sl, km_sb[:, kt:kt + 1])
                exps = []
                if d >= 1:
                    for h2 in range(2):
                        sl = slice(512 * h2 + d * P, 512 * h2 + 512)
                        exps.append(nc.scalar.activation(
                            E[:, 1024 * pr + 512 * h2 + d * P:
                               1024 * pr + 512 * h2 + 512], sim[:, sl], AF.Exp))
                else:
                    exps.append(nc.scalar.activation(
                        E[:, 1024 * pr:1024 * pr + 1024], sim, AF.Exp))
                # keep every Sqrt before every Exp in the ACT stream: the
                # scheduler otherwise interleaves them and forces repeated
                # activation-table reloads
                for e in exps:
                    for pa_i in prelude_acts:
                        add_dep(e.ins, pa_i.ins, True, "sqrt before exp")
            Es[kt] = E

        def emit_attention(qb, pieces, Es_pre=None):
            """pieces: phase-A closures spread evenly across the kt loop so PE
            has ready work while ACT churns exp. Es_pre: prefetched exp tiles
            (emitted during the previous block's ACT slack)."""
            nkt = 4 * qb + 4
            Es = dict(Es_pre or {})
            # spread pieces across the kt slots that actually emit sims (the
            # exp-paced ones) so PE filler lands where ACT is the pacer
            live = [kt for kt in range(nkt) if kt not in Es] or [nkt - 1]
            slots = [[] for _ in range(nkt)]
            for i, p in enumerate(pieces):
                slots[live[min(len(live) - 1,
                               i * len(live) // max(1, len(pieces)))]].append(p)

            for kt in range(nkt):
                d = kt - 4 * qb
                for p in slots[kt]:
                    p()
                if kt not in Es:
                    emit_sims(qb, kt, Es)
                if d >= 0:
                    # q-tile tb == d is complete: pv + normalize + out-proj
                    tb = d
                    qt = 4 * qb + tb
                    ppv = pb.tile([P, HPC * (DH + 1)], f32, name=f"ppv_{qt}",
                                  tag="ppv")
                    # one accumulation group at a time per bank: interleaved
                    # start/stop groups in a shared bank drop contributions
                    for pr in range(2):
                        for h2 in range(2):
                            hh = 2 * pr + h2
                            off = 1024 * pr + 512 * h2
                            for kt2 in range(qt + 1):
                                nc.tensor.matmul(
                                    ppv[:, 65 * hh:65 * hh + 65],
                                    Es[kt2][:, off + tb * P:off + (tb + 1) * P],
                                    v_sb[kt2][:, 65 * hh:65 * hh + 65],
                                    start=(kt2 == 0), stop=(kt2 == qt),
                                    skip_group_check=True)
                    rc = rcp.tile([P, HPC], f32, name=f"rc_{qt}", tag="rc")
                    pvw = ppv.rearrange("p (h c2) -> p h c2", c2=DH + 1)
                    nc.vector.reciprocal(rc, pvw[:, :, DH])
                    at = atp.tile([P, HPC * DH], bf16, name=f"at_{qt}", tag="at")
                    for hh in range(HPC):
                        nc.vector.tensor_scalar_mul(
                            at[:, DH * hh:DH * hh + DH],
                            ppv[:, 65 * hh:65 * hh + DH], rc[:, hh:hh + 1])
                    # transpose [tok, dims] -> [dims, tok] on PE, stage via Pool
                    tr = pb.tile([P, 2 * P], bf16, name=f"tr_{qt}", tag="po")
                    for m in range(2):
                        nc.tensor.transpose(tr[:, P * m:P * m + P],
                                            at[:, P * m:P * m + P], ident_sb)
                    atT = atp.tile([P, 2 * P], bf16, name=f"atT_{qt}", tag="atT")
                    nc.vector.tensor_copy(atT, tr)
                    for c2 in range(2):
                        po = pb.tile([P, 512], f32, name=f"po_{qt}_{c2}", tag="po")
                        for m in range(2):
                            nc.tensor.matmul(po, atT[:, P * m:P * m + P],
                                             wo_sb[m][:, 512 * c2:512 * c2 + 512],
                                             start=(m == 0), stop=(m == 1))
                        ob = obp.tile([P, 512], bf16, name=f"ob_{qt}_{c2}", tag="ob")
                        if c2 == 0:
                            nc.vector.tensor_copy(ob, po)
                        else:
                            nc.scalar.activation(ob, po, AF.Copy)
                        nc.sync.dma_start(
                            out=out_d[qt * P:(qt + 1) * P,
                                      c2 * 512:(c2 + 1) * 512],
                            in_=ob)
        # ---------------- main schedule -----------------------------------
        # preludes + chunk-0 projections first (Sqrt table phase), then
        # attention blocks with later projection chunks as PE filler.
        # Filler assignment respects deps: attn(j) needs qT(j) done up front,
        # kT(j)/v(j) only by its diagonal key tiles (kt >= 4j).
        prelude(0)
        c0 = qk_pieces(0, "q") + qk_pieces(0, "k") + v_pieces(0)
        for i, p in enumerate(c0):
            p()
            if i in (1, 3, 5):
                prelude(i // 2 + 1)
        emit_attention(0, qk_pieces(1, "q") + qk_pieces(1, "k"))
        emit_attention(1, v_pieces(1) + qk_pieces(2, "q") + qk_pieces(2, "k"))
        emit_attention(2, v_pieces(2) + qk_pieces(3, "q"))
        # prefetch qb3's first key tiles into dedicated E tiles: their exps run
        # in attn(2)'s ACT slack instead of pacing the final (piece-less) block
        Es3 = {}
        for kt in range(6):
            emit_sims(3, kt, Es3, tagp="p")
        emit_attention(3, qk_pieces(3, "k") + v_pieces(3), Es_pre=Es3)

    nc.compile()
    return nc


# ---------------------------------------------------------------- host side

def make_core_inputs(x, mask, pos_emb, g, Wq, Wkv, Wo, core, n):
    import ml_dtypes
    ndt = ml_dtypes.bfloat16
    b = core // 4
    h0 = (core % 4) * HPC
    scale = DH ** -0.5
    gW = Wq * g[:, None]
    gKV = Wkv * g[:, None]
    cols = slice(h0 * DH, (h0 + HPC) * DH)
    wq = gW[:, cols] * scale
    Wk_full = gKV[:, :D]
    wk = Wk_full[:, cols]
    wv = gKV[:, D:][:, cols]

    def rot_cols(W):
        # compact rotate-half sources; col-block order [h0|h2|h1|h3] so the
        # device-side u tiles land base-aligned with qT rot rows
        out = np.zeros((D, P), dtype=W.dtype)
        for b_, h in enumerate((0, 2, 1, 3)):
            src = W[:, (h0 + h) * DH:(h0 + h) * DH + DH]
            out[:, b_ * ROT:b_ * ROT + 16] = -src[:, 16:32]
            out[:, b_ * ROT + 16:b_ * ROT + 32] = src[:, 0:16]
        return out

    wqr = rot_cols(gW) * scale
    wkr = rot_cols(Wk_full)

    def pack_t(W):
        # [D, C] -> [128, KT*C] t-major
        C = W.shape[1]
        return np.ascontiguousarray(
            W.reshape(D // P, P, C).transpose(1, 0, 2).reshape(P, -1))

    cosf = np.cos(pos_emb.T).astype(np.float32)   # [32, n]
    sinf = np.sin(pos_emb.T).astype(np.float32)
    cos128 = np.ones((P, n), np.float32)
    cos128[0:ROT] = cosf
    cos128[DH:DH + ROT] = cosf
    sinc = np.zeros((P, n), np.float32)
    for h in range(HPC):
        sinc[h * ROT:(h + 1) * ROT] = sinf
    tri = np.where(np.arange(P)[:, None] <= np.arange(P)[None, :], 0.0, NEG
                   ).astype(np.float32)

    xT = np.ascontiguousarray(x[b].T)  # [D, n]
    ins = {
        "xT": pack_t(xT).astype(ndt),
        "wq": pack_t(wq).astype(ndt), "wk": pack_t(wk).astype(ndt),
        "wv": pack_t(wv).astype(ndt),
        "wqr": pack_t(wqr).astype(ndt), "wkr": pack_t(wkr).astype(ndt),
        "wo": Wo[cols, :].astype(ndt),
        "cos128": cos128.astype(ndt), "sinc": sinc.astype(ndt),
        "tri": tri, "ident": np.eye(P, dtype=ndt),
    }
    if not mask.all():
        km = np.where(mask[b], 0.0, NEG).astype(np.float32)
        ins["kmask"] = np.ascontiguousarray(km.reshape(n // P, P).T)
    return ins


# ---------------------------------------------------------------- runner

import os
import jax


def _run_per_device(nc, in_maps, core_ids):
    """Run the same Bass program independently on each visible device."""
    from concourse.bass2jax import (_bass_exec_p, install_neuronx_cc_hook,
                                    partition_id_tensor)
    install_neuronx_cc_hook()
    partition_name = nc.partition_id_tensor.name if nc.partition_id_tensor else None
    in_names, out_names, out_avals, zero_outs = [], [], [], []
    for alloc in nc.m.functions[0].allocations:
        if not isinstance(alloc, mybir.MemoryLocationSet):
            continue
        name = alloc.memorylocations[0].name
        if alloc.kind == "ExternalInput":
            if name != partition_name:
                in_names.append(name)
        elif alloc.kind == "ExternalOutput":
            out_names.append(name)
            shape = tuple(alloc.tensor_shape)
            dtype = mybir.dt.np(alloc.dtype)
            out_avals.append(jax.core.ShapedArray(shape, dtype))
            zero_outs.append(np.zeros(shape, dtype))
    n_params = len(in_names)
    all_in_names = list(in_names) + list(out_names)
    if partition_name is not None:
        all_in_names.append(partition_name)
    donate = tuple(range(n_params, n_params + len(out_names)))

    def _body(*args):
        operands = list(args)
        if partition_name is not None:
            operands.append(partition_id_tensor())
        outs = _bass_exec_p.bind(
            *operands, out_avals=tuple(out_avals), in_names=tuple(all_in_names),
            out_names=tuple(out_names), lowering_input_output_aliases=(),
            sim_require_finite=True, sim_require_nnan=True, nc=nc)
        return tuple(outs)

    fn = jax.jit(_body, donate_argnums=donate, keep_unused=True)
    futures = []
    for c, in_map in zip(core_ids, in_maps):
        dev = jax.devices()[c]
        args = [jax.device_put(np.asarray(in_map[nm]), dev) for nm in in_names]
        zz = [jax.device_put(z, dev) for z in zero_outs]
        futures.append(fn(*args, *zz))
    return [{nm: np.asarray(a) for nm, a in zip(out_names, f)} for f in futures]


_PROGRAM_CACHE = {}


def kernel(**inputs):
    os.environ.setdefault("NEURON_COMPILE_CACHE_URL", "/tmp/neuron_cache_kernel")
    x = np.asarray(inputs["x"], dtype=np.float32)
    mask = np.asarray(inputs["mask"]).astype(bool)
    pos_emb = np.asarray(inputs["pos_emb"], dtype=np.float32)
    g = np.asarray(inputs["g"], dtype=np.float32)
    Wq = np.asarray(inputs["Wq"], dtype=np.float32)
    Wkv = np.asarray(inputs["Wkv"], dtype=np.float32)
    Wo = np.asarray(inputs["Wo"], dtype=np.float32)
    bo = np.asarray(inputs["bo"], dtype=np.float32)
    b, n, _ = x.shape
    assert (b, n) == (2, 2048), (b, n)
    use_km = not bool(mask.all())
    key = (n, use_km)
    if key not in _PROGRAM_CACHE:
        _PROGRAM_CACHE[key] = build_program(n=n, use_kmask=use_km)
    nc = _PROGRAM_CACHE[key]
    core_ids = list(range(8))
    in_maps = [make_core_inputs(x, mask, pos_emb, g, Wq, Wkv, Wo, c, n)
               for c in core_ids]
    results = _run_per_device(nc, in_maps, core_ids)
    out = np.zeros((b, n, D), np.float32)
    for c in core_ids:
        out[c // 4] += results[c]["out"].astype(np.float32)
    out += bo[None, None, :]
    return out


# revision 13
# speedup vs baseline: 1.0001x; 1.0001x over previous
"""Sharded causal attention kernel for trn2, v3.

Sharding: 8 cores = 2 batches x 4 head-groups (4 heads each).
v3 vs v2:
  - all SBUF tensors bf16 (DVE 2x, half DMA); PSUM stays f32
  - software-pipelined: attention for q-block qb runs one chunk behind the
    projections, emission interleaved at key-tile granularity so PE always
    has ready work while ACT churns exp
  - compact rotary weights [D,128] (no zero columns)
  - rs_col via PE transposes of ssq (no diag-extract trick)
  - pv with v-as-moving (65-row matmuls), per-partition softmax normalize,
    DMA-xbar transpose of attn, then out projection
"""

from contextlib import ExitStack

import numpy as np

import concourse.bass as bass
import concourse.mybir as mybir
import concourse.tile as tile
from concourse import bacc
from concourse.bass import _add_dep_helper as add_dep

f32 = mybir.dt.float32
f32r = mybir.dt.float32r
bf16 = mybir.dt.bfloat16
AF = mybir.ActivationFunctionType
OP = mybir.AluOpType

D = 1024
HPC = 4          # heads per core
DH = 64
ROT = 32
P = 128
NEG = -1e30


def build_program(n=2048, use_kmask=False):
    KT = D // P          # 8 contraction tiles
    NCH = n // 512       # 4 token chunks
    NTOK = n // P        # 16 token tiles
    nc = bacc.Bacc("TRN2", target_bir_lowering=False, debug=False)

    def din(name, shape, dt_):
        return nc.dram_tensor(name, shape, dt_, kind="ExternalInput")

    # host packs x/weights t-major so each is one DMA into a [128, ...] tile
    xT_d = din("xT", [P, KT * n], bf16)
    wq_d = din("wq", [P, KT * HPC * DH], bf16)
    wk_d = din("wk", [P, KT * HPC * DH], bf16)
    wv_d = din("wv", [P, KT * HPC * DH], bf16)
    wqr_d = din("wqr", [P, KT * P], bf16)          # compact rot cols [4 heads x 32]
    wkr_d = din("wkr", [P, KT * P], bf16)
    wo_d = din("wo", [HPC * DH, D], bf16)
    cos_d = din("cos128", [P, n], bf16)            # qT-aligned: rows 0:32,64:96 cos
    sin_d = din("sinc", [P, n], bf16)              # compact: row 32h+j = sin_j
    tri_d = din("tri", [P, P], f32)
    id_d = din("ident", [P, P], bf16)
    km_d = din("kmask", [P, NTOK], f32) if use_kmask else None
    out_d = nc.dram_tensor("out", [n, D], bf16, kind="ExternalOutput")

    with tile.TileContext(nc) as tc, ExitStack() as top:
        persist = top.enter_context(tc.tile_pool(name="persist", bufs=1))
        ones_bf = persist.tile([P, 1], bf16, name="ones_bf")
        nc.vector.memset(ones_bf, 1.0)
        ones_row_f = persist.tile([1, P], f32, name="ones_row_f")
        nc.vector.memset(ones_row_f, 1.0)
        ones_row = persist.tile([1, P], f32r, name="ones_row")
        nc.vector.tensor_copy(ones_row, ones_row_f)
        tri_sb = persist.tile([P, P], f32, name="tri_sb")
        ident_sb = persist.tile([P, P], bf16, name="ident_sb")
        km_sb = persist.tile([P, NTOK], f32, name="km_sb") if use_kmask else None

        big = top.enter_context(tc.tile_pool(name="big", bufs=1))
        # x loaded chunk-major: one DMA brings all KT contraction tiles for a
        # 512-token chunk, so chunk-0 compute starts after ~1/4 of the x bytes
        x_all = big.tile([P, KT * n], bf16, name="x_all")
        x_allv = x_all.rearrange("p (t n) -> p t n", t=KT)
        xT_dv = xT_d.rearrange("p (t n) -> p t n", t=KT)
        x_sb = [x_all[:, t * n:(t + 1) * n] for t in range(KT)]
        wq_sb = big.tile([P, KT * HPC * DH], bf16, name="wq")
        wk_sb = big.tile([P, KT * HPC * DH], bf16, name="wk")
        wv_sb = big.tile([P, KT * HPC * DH], bf16, name="wv")
        wqr_sb = big.tile([P, KT * P], bf16, name="wqr")
        wkr_sb = big.tile([P, KT * P], bf16, name="wkr")
        cos_sb = big.tile([P, n], bf16, name="cos_sb")
        sin_sb = big.tile([P, n], bf16, name="sin_sb")
        wo_sb = [big.tile([P, D], bf16, name=f"wo{m}") for m in range(2)]
        # DMA issue order = single-queue service order: schedule each input
        # just before its first consumer needs it
        nc.sync.dma_start(out=x_allv[:, 0:2, 0:512], in_=xT_dv[:, 0:2, 0:512])
        nc.sync.dma_start(out=x_allv[:, 2:4, 0:512], in_=xT_dv[:, 2:4, 0:512])
        nc.sync.dma_start(out=x_allv[:, 4:KT, 0:512], in_=xT_dv[:, 4:KT, 0:512])
        nc.sync.dma_start(out=ident_sb, in_=id_d[:])
        nc.sync.dma_start(out=wq_sb, in_=wq_d[:])
        nc.sync.dma_start(out=wk_sb, in_=wk_d[:])
        nc.sync.dma_start(out=x_allv[:, :, 512:1024], in_=xT_dv[:, :, 512:1024])
        nc.sync.dma_start(out=wqr_sb, in_=wqr_d[:])
        nc.sync.dma_start(out=wkr_sb, in_=wkr_d[:])
        nc.sync.dma_start(out=cos_sb, in_=cos_d[:])
        nc.sync.dma_start(out=sin_sb, in_=sin_d[:])
        nc.sync.dma_start(out=x_allv[:, :, 1024:1536], in_=xT_dv[:, :, 1024:1536])
        nc.sync.dma_start(out=wv_sb, in_=wv_d[:])
        nc.sync.dma_start(out=x_allv[:, :, 1536:2048], in_=xT_dv[:, :, 1536:2048])
        nc.sync.dma_start(out=tri_sb, in_=tri_d[:])
        for m in range(2):
            nc.sync.dma_start(out=wo_sb[m], in_=wo_d[m * P:(m + 1) * P, :])
        if use_kmask:
            nc.sync.dma_start(out=km_sb, in_=km_d[:])
        wq = [wq_sb[:, t * HPC * DH:(t + 1) * HPC * DH] for t in range(KT)]
        wk = [wk_sb[:, t * HPC * DH:(t + 1) * HPC * DH] for t in range(KT)]
        wv = [wv_sb[:, t * HPC * DH:(t + 1) * HPC * DH] for t in range(KT)]
        wqr = [wqr_sb[:, t * P:(t + 1) * P] for t in range(KT)]
        wkr = [wkr_sb[:, t * P:(t + 1) * P] for t in range(KT)]

        qkv = top.enter_context(tc.tile_pool(name="qkv", bufs=1))
        qT = [qkv.tile([P, n], bf16, name=f"qT{m}", tag=f"qT{m}") for m in range(2)]
        kT = [qkv.tile([P, n], bf16, name=f"kT{m}", tag=f"kT{m}") for m in range(2)]
        v_sb = [qkv.tile([P, HPC * (DH + 1)], bf16, name=f"v{tk}", tag=f"v{tk}")
                for tk in range(NTOK)]
        rs_col = qkv.tile([P, NTOK], f32, name="rs_col")
        s_row = qkv.tile([1, n], f32r, name="s_row")
        ssq_sb = qkv.tile([1, n], bf16, name="ssq_sb")

        # PSUM budget (8 banks): pa (prelude + projections) 2,
        # pb (attention ppv accumulators + out-proj po) 2, psim 4
        pa = top.enter_context(tc.tile_pool(name="pa", bufs=2, space="PSUM"))
        pb = top.enter_context(tc.tile_pool(name="pb", bufs=1, space="PSUM"))
        psim = top.enter_context(tc.tile_pool(name="psim", bufs=1, space="PSUM"))
        sqp = top.enter_context(tc.tile_pool(name="sqp", bufs=2))
        rotu = top.enter_context(tc.tile_pool(name="rotu", bufs=2))
        esp = top.enter_context(tc.tile_pool(name="esp", bufs=1))
        atp = top.enter_context(tc.tile_pool(name="atp", bufs=2))
        obp = top.enter_context(tc.tile_pool(name="obp", bufs=2))
        rcp = top.enter_context(tc.tile_pool(name="rcp", bufs=2))

        # ---------------- phase A emitters (chunk c), as a piece list ----
        prelude_acts = []  # Sqrt instructions that must precede the first Exp

        def prelude(c):
            """rmsnorm stats for chunk c; all ACT Sqrt/Copy happen before the
            first Exp so the activation table loads exactly twice."""
            csl = slice(c * 512, (c + 1) * 512)
            ssq = pb.tile([1, 512], f32, name=f"ssq{c}", tag="ppv")
            for t in range(KT):
                sq = sqp.tile([P, 512], bf16, name=f"sq{t}_{c}", tag="sq")
                # spread squares across DVE/ACT/Pool (Square is in every act
                # table set; Pool is SBUF-only) to keep startup queues short
                if c == 0:
                    nc.vector.tensor_mul(sq, x_sb[t][:, csl], x_sb[t][:, csl])
                elif c == 2:
                    nc.scalar.activation(sq, x_sb[t][:, csl], AF.Square)
                else:
                    nc.gpsimd.tensor_mul(sq, x_sb[t][:, csl], x_sb[t][:, csl])
                nc.tensor.matmul(ssq, ones_bf, sq,
                                 start=(t == 0), stop=(t == KT - 1))
            # sqrt(ssq/D) then reciprocal (row form for bc broadcast)
            my_acts = []
            my_acts.append(
                nc.scalar.activation(s_row[:, csl], ssq, AF.Sqrt, scale=1.0 / D))
            with nc.allow_low_precision(reason="f32r is f32-width"):
                nc.vector.reciprocal(s_row[:, csl], s_row[:, csl])
            my_acts.append(
                nc.scalar.activation(ssq_sb[:, csl], ssq, AF.Copy))
            # rs_col tiles via PE transpose of ssq (bf16: f32r transpose is
            # broken in walrus codegen)
            rst = pb.tile([P, 8], bf16, name=f"rst{c}", tag="ppv")
            rstv = rst.rearrange("p (a b) -> p a b", b=2)
            for tb in range(4):
                tk = c * 4 + tb
                # even columns only: PSUM accesses must be 4-byte aligned
                nc.tensor.transpose(rstv[:, tb, 0:1],
                                    ssq_sb[:, tk * P:(tk + 1) * P],
                                    ones_bf[0:1, 0:1])
            my_acts.append(
                nc.scalar.activation(rs_col[:, c * 4:c * 4 + 4], rstv[:, :, 0],
                                     AF.Sqrt, scale=1.0 / D))
            if c < NCH - 1:
                # c3's stats come late (Pool squares); gating every exp on it
                # would stall attention — let it float and eat one table reload
                prelude_acts.extend(my_acts)
            nc.vector.reciprocal(rs_col[:, c * 4:c * 4 + 4],
                                 rs_col[:, c * 4:c * 4 + 4])
            # broadcast rs over partitions, fold into cos/sin
            bc = pb.tile([P, 512], f32, name=f"bc{c}", tag="ppv")
            nc.tensor.matmul(bc, ones_row, s_row[:, csl],
                             start=True, stop=True)
            nc.vector.tensor_mul(cos_sb[:, csl], cos_sb[:, csl], bc)
            nc.vector.tensor_mul(sin_sb[:, csl], sin_sb[:, csl], bc)

        def qk_pieces(c, which):
            csl = slice(c * 512, (c + 1) * 512)
            base, wmain, wrot, nm_ = ((qT, wq, wqr, "q") if which == "q"
                                      else (kT, wk, wkr, "k"))

            def p_m(m):
                ps = pa.tile([P, 512], f32, name=f"p{nm_}{m}_{c}", tag="pa")
                for t in range(KT):
                    nc.tensor.matmul(ps, wmain[t][:, m * P:(m + 1) * P],
                                     x_sb[t][:, csl],
                                     start=(t == 0), stop=(t == KT - 1))
                nc.vector.tensor_mul(base[m][:, csl], ps, cos_sb[:, csl])

            def p_rot():
                # psr partition layout (wqr col order [h0|h2|h1|h3]):
                # m=0 reads rows 0:96 (h0,-,h1), m=1 rows 32:128 (h2,-,h3);
                # u tiles land base-aligned with qT rot rows {0:32, 64:96} so
                # the SBUF-SBUF adds have equal base partitions (hw rule)
                psr = pa.tile([P, 512], f32, name=f"p{nm_}r_{c}", tag="pa")
                for t in range(KT):
                    nc.tensor.matmul(psr, wrot[t], x_sb[t][:, csl],
                                     start=(t == 0), stop=(t == KT - 1))
                for m in range(2):
                    u = rotu.tile([P, 512], bf16, name=f"u_{nm_}{m}_{c}", tag="u")
                    if m == 0:
                        nc.vector.tensor_mul(u[0:96, :], psr[0:96, :],
                                             sin_sb[0:96, csl])
                    else:
                        # aligned partition windows: <=32 parts from base 32/96
                        nc.vector.tensor_mul(u[0:32, :], psr[32:64, :],
                                             sin_sb[0:32, csl])
                        nc.vector.tensor_mul(u[64:96, :], psr[96:128, :],
                                             sin_sb[64:96, csl])
                    for h2 in range(2):
                        bsl = base[m][64 * h2:64 * h2 + 32, csl]
                        usl = u[64 * h2:64 * h2 + 32, :]
                        if c == 0:
                            nc.vector.tensor_tensor(bsl, bsl, usl, OP.add)
                        else:
                            nc.gpsimd.tensor_tensor(bsl, bsl, usl, OP.add)

            return [lambda: p_m(0), lambda: p_m(1), p_rot]

        def v_pieces(c):
            out = []
            for tb in range(4):
                tk = c * 4 + tb

                def p_v(tk=tk):
                    pv = pa.tile([P, HPC * DH], f32, name=f"pv_{tk}", tag="pa")
                    for t in range(KT):
                        nc.tensor.matmul(pv, x_sb[t][:, tk * P:(tk + 1) * P],
                                         wv[t], start=(t == 0), stop=(t == KT - 1))
                    vv = v_sb[tk].rearrange("p (h c2) -> p h c2", h=HPC)
                    nc.vector.tensor_scalar_mul(
                        vv[:, :, 0:DH], pv.rearrange("p (h c2) -> p h c2", h=HPC),
                        rs_col[:, tk:tk + 1])
                    for hh in range(HPC):
                        nc.gpsimd.tensor_copy(vv[:, hh, DH:DH + 1], ones_bf)

                out.append(p_v)
            return out

        # ---------------- attention emitter for q-block qb -----------------
        def emit_sims(qb, kt, Es, tagp=""):
            """sims + mask + exp for one key tile (all 4 heads in one 4-bank
            psum tile -> single exp op per off-diag tile)."""
            d = kt - 4 * qb
            lo = max(0, d) * P  # q cols < lo are strictly above the diag
            E = esp.tile([P, 2048], bf16, name=f"E{tagp}_{kt}",
                         tag=f"E{tagp}_{kt}")
            for pr in range(2):
                sim = psim.tile([P, 1024], f32, name=f"s{tagp}{pr}_{qb}_{kt}",
                                tag=f"sim{pr}")
                for h2 in range(2):
                    nc.tensor.matmul(
                        sim[:, 512 * h2 + lo:512 * h2 + 512],
                        kT[pr][64 * h2:64 * h2 + 64, kt * P:(kt + 1) * P],
                        qT[pr][64 * h2:64 * h2 + 64,
                               qb * 512 + lo:(qb + 1) * 512],
                        start=True, stop=True, tile_position=(64 * h2, 0))
                if d >= 0:
                    for h2 in range(2):
                        sl = sim[:, 512 * h2 + d * P:512 * h2 + (d + 1) * P]
                        nc.vector.tensor_tensor(sl, sl, tri_sb, OP.add)
                if use_kmask:
                    for h2 in range(2):
                        sl = sim[:, 512 * h2:512 * h2 + 512]
                        nc.vector.tensor_scalar_add(sl, sl, km_sb[:, kt:kt + 1])
                exps = []
                if d >= 1:
                    for h2 in range(2):
                        sl = slice(512 * h2 + d * P, 512 * h2 + 512)
                        exps.append(nc.scalar.activation(
                            E[:, 1024 * pr + 512 * h2 + d * P:
                               1024 * pr + 512 * h2 + 512], sim[:, sl], AF.Exp))
                else:
                    exps.append(nc.scalar.activation(
                        E[:, 1024 * pr:1024 * pr + 1024], sim, AF.Exp))
                # keep every Sqrt before every Exp in the ACT stream: the
                # scheduler otherwise interleaves them and forces repeated
                # activation-table reloads
                for e in exps:
                    for pa_i in prelude_acts:
                        add_dep(e.ins, pa_i.ins, True, "sqrt before exp")
            Es[kt] = E

        def emit_attention(qb, pieces, Es_pre=None):
            """pieces: phase-A closures spread evenly across the kt loop so PE
            has ready work while ACT churns exp. Es_pre: prefetched exp tiles
            (emitted during the previous block's ACT slack)."""
            nkt = 4 * qb + 4
            Es = dict(Es_pre or {})
            # spread pieces across the kt slots that actually emit sims (the
            # exp-paced ones) so PE filler lands where ACT is the pacer
            live = [kt for kt in range(nkt) if kt not in Es] or [nkt - 1]
            slots = [[] for _ in range(nkt)]
            for i, p in enumerate(pieces):
                slots[live[min(len(live) - 1,
                               i * len(live) // max(1, len(pieces)))]].append(p)

            for kt in range(nkt):
                d = kt - 4 * qb
                for p in slots[kt]:
                    p()
                if kt not in Es:
                    emit_sims(qb, kt, Es)
                if d >= 0:
                    # q-tile tb == d is complete: pv + normalize + out-proj
                    tb = d
                    qt = 4 * qb + tb
                    ppv = pb.tile([P, HPC * (DH + 1)], f32, name=f"ppv_{qt}",
                                  tag="ppv")
                    # one accumulation group at a time per bank: interleaved
                    # start/stop groups in a shared bank drop contributions
                    for pr in range(2):
                        for h2 in range(2):
                            hh = 2 * pr + h2
                            off = 1024 * pr + 512 * h2
                            for kt2 in range(qt + 1):
                                nc.tensor.matmul(
                                    ppv[:, 65 * hh:65 * hh + 65],
                                    Es[kt2][:, off + tb * P:off + (tb + 1) * P],
                                    v_sb[kt2][:, 65 * hh:65 * hh + 65],
                                    start=(kt2 == 0), stop=(kt2 == qt),
                                    skip_group_check=True)
                    rc = rcp.tile([P, HPC], f32, name=f"rc_{qt}", tag="rc")
                    pvw = ppv.rearrange("p (h c2) -> p h c2", c2=DH + 1)
                    nc.vector.reciprocal(rc, pvw[:, :, DH])
                    at = atp.tile([P, HPC * DH], bf16, name=f"at_{qt}", tag="at")
                    for hh in range(HPC):
                        nc.vector.tensor_scalar_mul(
                            at[:, DH * hh:DH * hh + DH],
                            ppv[:, 65 * hh:65 * hh + DH], rc[:, hh:hh + 1])
                    # transpose [tok, dims] -> [dims, tok] on PE, stage via Pool
                    tr = pb.tile([P, 2 * P], bf16, name=f"tr_{qt}", tag="po")
                    for m in range(2):
                        nc.tensor.transpose(tr[:, P * m:P * m + P],
                                            at[:, P * m:P * m + P], ident_sb)
                    atT = atp.tile([P, 2 * P], bf16, name=f"atT_{qt}", tag="atT")
                    nc.vector.tensor_copy(atT, tr)
                    for c2 in range(2):
                        po = pb.tile([P, 512], f32, name=f"po_{qt}_{c2}", tag="po")
                        for m in range(2):
                            nc.tensor.matmul(po, atT[:, P * m:P * m + P],
                                             wo_sb[m][:, 512 * c2:512 * c2 + 512],
                                             start=(m == 0), stop=(m == 1))
                        ob = obp.tile([P, 512], bf16, name=f"ob_{qt}_{c2}", tag="ob")
                        if c2 == 0:
                            nc.vector.tensor_copy(ob, po)
                        else:
                            nc.scalar.activation(ob, po, AF.Copy)
                        nc.sync.dma_start(
                            out=out_d[qt * P:(qt + 1) * P,
                                      c2 * 512:(c2 + 1) * 512],
                            in_=ob)
        # ---------------- main schedule -----------------------------------
        # preludes + chunk-0 projections first (Sqrt table phase), then
        # attention blocks with later projection chunks as PE filler.
        # Filler assignment respects deps: attn(j) needs qT(j) done up front,
        # kT(j)/v(j) only by its diagonal key tiles (kt >= 4j).
        prelude(0)
        c0 = qk_pieces(0, "q") + qk_pieces(0, "k") + v_pieces(0)
        for i, p in enumerate(c0):
            p()
            if i in (1, 3, 5):
                prelude(i // 2 + 1)
        emit_attention(0, qk_pieces(1, "q") + qk_pieces(1, "k"))
        emit_attention(1, v_pieces(1) + qk_pieces(2, "q") + qk_pieces(2, "k"))
        emit_attention(2, v_pieces(2) + qk_pieces(3, "q"))
        # prefetch qb3's first key tiles into dedicated E tiles: their exps run
        # in attn(2)'s ACT slack instead of pacing the final (piece-less) block
        Es3 = {}
        for kt in range(6):
            emit_sims(3, kt, Es3, tagp="p")
        emit_attention(3, qk_pieces(3, "k") + v_pieces(3), Es_pre=Es3)

    nc.compile()
    return nc


# ---------------------------------------------------------------- host side

def make_core_inputs(x, mask, pos_emb, g, Wq, Wkv, Wo, core, n):
    import ml_dtypes
    ndt = ml_dtypes.bfloat16
    b = core // 4
    h0 = (core % 4) * HPC
    scale = DH ** -0.5
    gW = Wq * g[:, None]
    gKV = Wkv * g[:, None]
    cols = slice(h0 * DH, (h0 + HPC) * DH)
    wq = gW[:, cols] * scale
    Wk_full = gKV[:, :D]
    wk = Wk_full[:, cols]
    wv = gKV[:, D:][:, cols]

    def rot_cols(W):
        # compact rotate-half sources; col-block order [h0|h2|h1|h3] so the
        # device-side u tiles land base-aligned with qT rot rows
        out = np.zeros((D, P), dtype=W.dtype)
        for b_, h in enumerate((0, 2, 1, 3)):
            src = W[:, (h0 + h) * DH:(h0 + h) * DH + DH]
            out[:, b_ * ROT:b_ * ROT + 16] = -src[:, 16:32]
            out[:, b_ * ROT + 16:b_ * ROT + 32] = src[:, 0:16]
        return out

    wqr = rot_cols(gW) * scale
    wkr = rot_cols(Wk_full)

    def pack_t(W):
        # [D, C] -> [128, KT*C] t-major
        C = W.shape[1]
        return np.ascontiguousarray(
            W.reshape(D // P, P, C).transpose(1, 0, 2).reshape(P, -1))

    cosf = np.cos(pos_emb.T).astype(np.float32)   # [32, n]
    sinf = np.sin(pos_emb.T).astype(np.float32)
    cos128 = np.ones((P, n), np.float32)
    cos128[0:ROT] = cosf
    cos128[DH:DH + ROT] = cosf
    sinc = np.zeros((P, n), np.float32)
    for h in range(HPC):
        sinc[h * ROT:(h + 1) * ROT] = sinf
    tri = np.where(np.arange(P)[:, None] <= np.arange(P)[None, :], 0.0, NEG
                   ).astype(np.float32)

    xT = np.ascontiguousarray(x[b].T)  # [D, n]
    ins = {
        "xT": pack_t(xT).astype(ndt),
        "wq": pack_t(wq).astype(ndt), "wk": pack_t(wk).astype(ndt),
        "wv": pack_t(wv).astype(ndt),
        "wqr": pack_t(wqr).astype(ndt), "wkr": pack_t(wkr).astype(ndt),
        "wo": Wo[cols, :].astype(ndt),
        "cos128": cos128.astype(ndt), "sinc": sinc.astype(ndt),
        "tri": tri, "ident": np.eye(P, dtype=ndt),
    }
    if not mask.all():
        km = np.where(mask[b], 0.0, NEG).astype(np.float32)
        ins["kmask"] = np.ascontiguousarray(km.reshape(n // P, P).T)
    return ins


# ---------------------------------------------------------------- runner

import os
import jax


def _run_per_device(nc, in_maps, core_ids):
    """Run the same Bass program independently on each visible device."""
    from concourse.bass2jax import (_bass_exec_p, install_neuronx_cc_hook,
                                    partition_id_tensor)
    install_neuronx_cc_hook()
    partition_name = nc.partition_id_tensor.name if nc.partition_id_tensor else None
    in_names, out_names, out_avals, zero_outs = [], [], [], []
    for alloc in nc.m.functions[0].allocations:
        if not isinstance(alloc, mybir.MemoryLocationSet):
            continue
        name = alloc.memorylocations[0].name
        if alloc.kind == "ExternalInput":
            if name != partition_name:
                in_names.append(name)
        elif alloc.kind == "ExternalOutput":
            out_names.append(name)
            shape = tuple(alloc.tensor_shape)
            dtype = mybir.dt.np(alloc.dtype)
            out_avals.append(jax.core.ShapedArray(shape, dtype))
            zero_outs.append(np.zeros(shape, dtype))
    n_params = len(in_names)
    all_in_names = list(in_names) + list(out_names)
    if partition_name is not None:
        all_in_names.append(partition_name)
    donate = tuple(range(n_params, n_params + len(out_names)))

    def _body(*args):
        operands = list(args)
        if partition_name is not None:
            operands.append(partition_id_tensor())
        outs = _bass_exec_p.bind(
            *operands, out_avals=tuple(out_avals), in_names=tuple(all_in_names),
            out_names=tuple(out_names), lowering_input_output_aliases=(),
            sim_require_finite=True, sim_require_nnan=True, nc=nc)
        return tuple(outs)

    fn = jax.jit(_body, donate_argnums=donate, keep_unused=True)
    futures = []
    for c, in_map in zip(core_ids, in_maps):
        dev = jax.devices()[c]
        args = [jax.device_put(np.asarray(in_map[nm]), dev) for nm in in_names]
        zz = [jax.device_put(z, dev) for z in zero_outs]
        futures.append(fn(*args, *zz))
    return [{nm: np.asarray(a) for nm, a in zip(out_names, f)} for f in futures]


_PROGRAM_CACHE = {}


def kernel(**inputs):
    os.environ.setdefault("NEURON_COMPILE_CACHE_URL", "/tmp/neuron_cache_kernel")
    x = np.asarray(inputs["x"], dtype=np.float32)
    mask = np.asarray(inputs["mask"]).astype(bool)
    pos_emb = np.asarray(inputs["pos_emb"], dtype=np.float32)
    g = np.asarray(inputs["g"], dtype=np.float32)
    Wq = np.asarray(inputs["Wq"], dtype=np.float32)
    Wkv = np.asarray(inputs["Wkv"], dtype=np.float32)
    Wo = np.asarray(inputs["Wo"], dtype=np.float32)
    bo = np.asarray(inputs["bo"], dtype=np.float32)
    b, n, _ = x.shape
    assert (b, n) == (2, 2048), (b, n)
    use_km = not bool(mask.all())
    key = (n, use_km)
    if key not in _PROGRAM_CACHE:
        _PROGRAM_CACHE[key] = build_program(n=n, use_kmask=use_km)
    nc = _PROGRAM_CACHE[key]
    core_ids = list(range(8))
    in_maps = [make_core_inputs(x, mask, pos_emb, g, Wq, Wkv, Wo, c, n)
               for c in core_ids]
    results = _run_per_device(nc, in_maps, core_ids)
    out = np.zeros((b, n, D), np.float32)
    for c in core_ids:
        out[c // 4] += results[c]["out"].astype(np.float32)
    out += bo[None, None, :]
    return out
